# revision 1
# baseline (speedup 1.0000x reference)
"""AudioMamba2 fused TRN2 kernel: 8-core data-parallel Bass/Tile.

Row-major, 5-stage software-pipelined macro loop (64 blocks of 128 rows
per macro; stages A=MM1/silu/t1/bc, B=y/transpose/ss, C=MM2/scale,
exp/softmax-sum, normalize/out-DMA run one macro apart and are emitted
slot-interleaved so every engine stream alternates ready work with
freshly-dependent work).

Key techniques:
- host-folded weights (f_out+in_proj+conv collapse into one [37,152]
  bf16 matmul with a baked ones-row bias column)
- d-major permutation of the 64 inner channels so the per-head f
  broadcast multiply hits the DVE 2x_1p packed-bf16 mode
- paired MM2 via a block-diagonal [[W',0],[0,W']] moving operand
  (one matmul per transposed 2-block pair, row-major output)
- sum-of-squares via PE matmuls with zero-padded per-subgroup selector
  columns accumulating into one PSUM tile (start/stop), transposed
  back to row-major for one batched rsqrt (Ln+Exp) on ACT
- engine placement honoring TRN2 limits: Pool/GPSIMD never touches
  PSUM, tensor_tensor never reads bf16 PSUM (tensor_copy may)
"""
import numpy as np
import contextlib
import ml_dtypes

import concourse.bass as bass
import concourse.mybir as mybir
import concourse.tile as tile
from concourse.bass_types import AP

F32 = mybir.dt.float32
BF16 = mybir.dt.bfloat16
AF = mybir.ActivationFunctionType
ALU = mybir.AluOpType

IN_DIM = 36
D_MODEL = 32
D_INNER = 64
NHEADS = 8
D_IN_PROJ = 152
NORM_EPS = 1e-5
K1 = 37          # 36 features + ones row
NSIL = 144       # z|xh|B|C channels (silu'd)

MACRO = 64       # blocks per macro-iteration
GROUPS = (7, 7, 7, 7, 7, 7, 7, 7, 4, 4)   # MM1/silu groups inside a macro


def fold_weights(f_out_w, f_out_b, in_proj_w, conv_w, conv_b, dt_bias,
                 A_log, D_skip, norm_w, out_proj_w):
    f64 = np.float64
    W12 = in_proj_w.astype(f64) @ f_out_w.astype(f64)          # [152, 36]
    b12 = in_proj_w.astype(f64) @ f_out_b.astype(f64)          # [152]
    s80 = conv_w[:, -1].astype(f64)
    W12[64:144] *= s80[:, None]
    b12[64:144] = b12[64:144] * s80 + conv_b.astype(f64)
    b12[144:152] += dt_bias.astype(f64)
    W1 = np.concatenate([W12, b12[:, None]], axis=1)           # [152, 37]
    Wout = out_proj_w.astype(f64) * norm_w.astype(f64)[None, :]  # [32, 64]
    WoutT = np.ascontiguousarray(Wout.T)                       # [64, 32]
    # d-major permutation of the 64 inner channels: new j=(d*8+h) <- old h*8+d
    perm = np.array([(j % 8) * 8 + j // 8 for j in range(64)])
    W1p = W1.copy()
    W1p[0:64] = W1[0:64][perm]          # z block
    W1p[64:128] = W1[64:128][perm]      # xh block
    WoutTp = WoutT[perm]                # rows follow the y channel order
    W1T = np.ascontiguousarray(W1p.T)                          # [37, 152]
    WoutT4 = np.zeros((128, 64), np.float64)   # [[W', 0], [0, W']]
    WoutT4[0:64, 0:32] = WoutTp
    WoutT4[64:128, 32:64] = WoutTp
    return (W1T.astype(ml_dtypes.bfloat16),
            WoutT4.astype(ml_dtypes.bfloat16),
            np.ascontiguousarray(
                np.broadcast_to(D_skip.astype(ml_dtypes.bfloat16),
                                (128, 8))))


def prep_xt(x):
    """x [N, 36] f32 -> xT [37, N] bf16 with ones row."""
    N = x.shape[0]
    xt = np.empty((K1, N), dtype=ml_dtypes.bfloat16)
    xt[:IN_DIM] = x.T.astype(ml_dtypes.bfloat16)
    xt[IN_DIM] = np.float32(1.0)
    return xt


def make_consts():
    essq = np.zeros((128, 8, 16), dtype=ml_dtypes.bfloat16)
    for q in range(8):
        essq[0:64, q, q] = 1.0          # parity 0 rows -> col q
        essq[64:128, q, 8 + q] = 1.0    # parity 1 rows -> col 8+q
    essq = np.ascontiguousarray(essq.reshape(128, 128))
    ident = np.eye(128, dtype=ml_dtypes.bfloat16)
    return essq, ident


def build_kernel(npc, num_cores=8, sim_safe=False):
    assert npc % (MACRO * 128) == 0
    NB = npc // 128
    NM = NB // MACRO
    nc = bass.Bass("TRN2", target_bir_lowering=False, num_devices=num_cores)

    xt_d = nc.dram_tensor("xt", [K1, npc], BF16, kind="ExternalInput")
    w1t_d = nc.dram_tensor("w1t", [K1, D_IN_PROJ], BF16, kind="ExternalInput")
    woutt_d = nc.dram_tensor("woutt", [128, 2 * D_MODEL], BF16,
                             kind="ExternalInput")
    d_d = nc.dram_tensor("dskip", [128, NHEADS], BF16, kind="ExternalInput")
    essq_d = nc.dram_tensor("essq", [128, 128], BF16, kind="ExternalInput")
    id_d = nc.dram_tensor("ident", [128, 128], BF16, kind="ExternalInput")
    idf_d = nc.dram_tensor("identf", [16, 16], F32, kind="ExternalInput")
    eps_d = nc.dram_tensor("eps", [128, 1], F32, kind="ExternalInput")
    out_d = nc.dram_tensor("out", [128, NB * D_MODEL], F32,
                           kind="ExternalOutput")

    # persistent SBUF constants
    w1t_s = nc.alloc_sbuf_tensor("w1t_s", [K1, D_IN_PROJ], BF16)
    woutt_s = nc.alloc_sbuf_tensor("woutt_s", [128, 2 * D_MODEL], BF16)
    d_s = nc.alloc_sbuf_tensor("d_s", [128, NHEADS], BF16)
    essq_s = nc.alloc_sbuf_tensor("essq_s", [128, 128], BF16)
    id_s = nc.alloc_sbuf_tensor("id_s", [128, 128], BF16)
    idf_s = nc.alloc_sbuf_tensor("idf_s", [16, 16], F32)
    eps_s = nc.alloc_sbuf_tensor("eps_s", [128, 1], F32)

    # manual PSUM map (8 banks exactly)
    pa = [nc.alloc_psum_tensor("pa0", [128, 1024], F32),
          nc.alloc_psum_tensor("pa1", [128, 1024], F32)]    # 2 banks each
    pb = nc.alloc_psum_tensor("pb", [128, 512], F32)        # dt preacts
    ssb = nc.alloc_psum_tensor("ssb", [128, 512], F32)      # ss area + sstr
    ytp = nc.alloc_psum_tensor("ytp", [128, 1024], BF16)    # 2 half-buffers
    o2 = nc.alloc_psum_tensor("o2", [128, 512], F32)        # 2 halves of 8 blk

    ve = nc.vector
    ge = nc.gpsimd

    def silu(out_ap, in_ap):
        if sim_safe:
            nc.scalar.activation(out_ap, in_ap, AF.Sigmoid)
            ve.tensor_tensor(out=out_ap, in0=out_ap, in1=in_ap, op=ALU.mult)
        else:
            nc.scalar.activation(out_ap, in_ap, AF.Silu)

    def sub(ap, off, dims):
        """AP at free-element offset `off` with explicit free dims."""
        return AP(ap.tensor, ap.offset + off, [list(ap.ap[0])] + dims)

    with tile.TileContext(nc) as tc:
        nc.sync.dma_start(w1t_s.ap(), w1t_d.ap())
        nc.scalar.dma_start(id_s.ap(), id_d.ap())
        nc.scalar.dma_start(woutt_s.ap(), woutt_d.ap())
        nc.scalar.dma_start(d_s.ap(), d_d.ap())
        nc.scalar.dma_start(essq_s.ap(), essq_d.ap())
        nc.scalar.dma_start(idf_s.ap(), idf_d.ap())
        nc.scalar.dma_start(eps_s.ap(), eps_d.ap())

        with contextlib.ExitStack() as _ctx:
            xtp = _ctx.enter_context(tc.tile_pool(name="xtp", bufs=3))
            sp = _ctx.enter_context(tc.tile_pool(name="sp", bufs=4))
            t1p = _ctx.enter_context(tc.tile_pool(name="t1p", bufs=2))
            bcpp = _ctx.enter_context(tc.tile_pool(name="bcpp", bufs=4))
            bcsp = _ctx.enter_context(tc.tile_pool(name="bcsp", bufs=3))
            dtp = _ctx.enter_context(tc.tile_pool(name="dtp", bufs=3))
            fp = _ctx.enter_context(tc.tile_pool(name="fp", bufs=3))
            yup = _ctx.enter_context(tc.tile_pool(name="yup", bufs=8))
            ytsp = _ctx.enter_context(tc.tile_pool(name="ytsp", bufs=3))
            sqp = _ctx.enter_context(tc.tile_pool(name="sqp", bufs=4))
            ssbp = _ctx.enter_context(tc.tile_pool(name="ssbp", bufs=3))
            rp = _ctx.enter_context(tc.tile_pool(name="rp", bufs=3))
            onp = _ctx.enter_context(tc.tile_pool(name="onp", bufs=3))
            ep = _ctx.enter_context(tc.tile_pool(name="ep", bufs=3))
            sep = _ctx.enter_context(tc.tile_pool(name="sep", bufs=4))
            se1p = _ctx.enter_context(tc.tile_pool(name="se1p", bufs=2))
            se2p = _ctx.enter_context(tc.tile_pool(name="se2p", bufs=2))
            osp = _ctx.enter_context(tc.tile_pool(name="osp", bufs=2))
            states = {}

            def emit_mm2_group(st, hh, part="both"):
                """Paired MM2s for 8 blocks + the r-scale multiply."""
                oh = (hh % 2) * 256
                if part in ("both", "mm"):
                    for pair in range(4):
                        lhs = st["yts"][:, hh * 512 + pair * 128:
                                        hh * 512 + (pair + 1) * 128]
                        nc.tensor.matmul(
                            o2[:, oh + pair * 64:oh + (pair + 1) * 64],
                            lhs, woutt_s.ap())
                if part in ("both", "on"):
                    # o2 cols (pair, par, ch); r column u = 16c + 8par + q
                    ve.tensor_tensor(
                        out=sub(st["on"], hh * 256, [[32, 8], [1, 32]]),
                        in0=sub(o2.ap(), oh, [[64, 4], [32, 2], [1, 32]]),
                        in1=sub(st["r"], hh, [[16, 4], [8, 2], [0, 32]]),
                        op=ALU.mult)

            def emit_c_exp(st):
                e_t = ep.tile([128, MACRO * D_MODEL], BF16)
                nc.scalar.activation(e_t, st["on"], AF.Exp)
                se1_t = se1p.tile([128, MACRO * 16], BF16)
                ve.tensor_tensor(
                    out=se1_t,
                    in0=sub(e_t, 0, [[32, MACRO], [1, 16]]),
                    in1=sub(e_t, 16, [[32, MACRO], [1, 16]]), op=ALU.add)
                se2_t = se2p.tile([128, MACRO * 8], BF16)
                ve.tensor_tensor(
                    out=se2_t,
                    in0=sub(se1_t, 0, [[16, MACRO], [1, 8]]),
                    in1=sub(se1_t, 8, [[16, MACRO], [1, 8]]), op=ALU.add)
                se3_t = se1p.tile([128, MACRO * 4], BF16)
                ve.tensor_tensor(
                    out=se3_t,
                    in0=sub(se2_t, 0, [[8, MACRO], [1, 4]]),
                    in1=sub(se2_t, 4, [[8, MACRO], [1, 4]]), op=ALU.add)
                se4_t = se2p.tile([128, MACRO * 2], BF16)
                ve.tensor_tensor(
                    out=se4_t,
                    in0=sub(se3_t, 0, [[4, MACRO], [1, 2]]),
                    in1=sub(se3_t, 2, [[4, MACRO], [1, 2]]), op=ALU.add)
                se_t = sep.tile([128, MACRO], F32)
                ve.tensor_tensor(
                    out=se_t,
                    in0=sub(se4_t, 0, [[2, MACRO]]),
                    in1=sub(se4_t, 1, [[2, MACRO]]), op=ALU.add)
                st["e"], st["se"] = e_t, se_t

            def emit_c_norm(st):
                rec_t = sep.tile([128, MACRO], F32)
                ve.reciprocal(rec_t, st["se"])
                os_t = osp.tile([128, MACRO * D_MODEL], F32)
                ge.tensor_tensor(
                    out=os_t, in0=st["e"],
                    in1=sub(rec_t, 0, [[1, MACRO], [0, D_MODEL]]),
                    op=ALU.mult)
                nc.sync.dma_start(
                    out_d[:, st["mb0"] * D_MODEL:
                          (st["mb0"] + MACRO) * D_MODEL],
                    os_t)

            def emit_a_mms(st, gi, g0, G):
                pa_t = pa[gi % 2]
                xt_t = st["xt"]
                for j in range(G):
                    b = g0 + j
                    xt_sl = xt_t[:, b * 128:(b + 1) * 128]
                    off = j * NSIL
                    if off < 512 < off + NSIL:
                        cut = 512 - off
                        nc.tensor.matmul(pa_t[:, off:512],
                                         xt_sl, w1t_s[:, 0:cut])
                        nc.tensor.matmul(pa_t[:, 512:off + NSIL],
                                         xt_sl, w1t_s[:, cut:NSIL])
                    else:
                        nc.tensor.matmul(pa_t[:, off:off + NSIL],
                                         xt_sl, w1t_s[:, 0:NSIL])
                    nc.tensor.matmul(pb[:, b * 8:b * 8 + 8],
                                     xt_sl, w1t_s[:, NSIL:D_IN_PROJ])

            def emit_a_group(st, gi, g0, G):
                pa_t = pa[gi % 2]
                t1_t, bc_t = st["t1"], st["bc"]
                s_t = sp.tile([128, 1008], BF16)
                silu(sub(s_t, 0, [[NSIL, G], [1, NSIL]]),
                     sub(pa_t.ap(), 0, [[NSIL, G], [1, NSIL]]))
                ve.tensor_tensor(
                    out=sub(t1_t, g0 * D_INNER,
                            [[D_INNER, G], [1, D_INNER]]),
                    in0=sub(s_t, 0, [[NSIL, G], [1, 64]]),
                    in1=sub(s_t, 64, [[NSIL, G], [1, 64]]),
                    op=ALU.mult)
                bcp_t = bcpp.tile([128, 14 * 8], BF16)
                ge.tensor_tensor(
                    out=sub(bcp_t, 0, [[8, G], [1, 8]]),
                    in0=sub(s_t, 128, [[NSIL, G], [1, 8]]),
                    in1=sub(s_t, 136, [[NSIL, G], [1, 8]]),
                    op=ALU.mult)
                ve.tensor_reduce(
                    out=sub(bc_t, g0, [[1, G]]),
                    in_=sub(bcp_t, 0, [[8, G], [1, 8]]),
                    axis=mybir.AxisListType.X, op=ALU.add)

            def emit_a_tail(st):
                dt_t = dtp.tile([128, MACRO * NHEADS], BF16)
                nc.scalar.activation(dt_t, pb.ap(), AF.Exp)
                nc.scalar.activation(dt_t, dt_t, AF.Ln, bias=1.0)
                f_t = fp.tile([128, MACRO * NHEADS], BF16)
                ge.tensor_tensor(
                    out=f_t, in0=dt_t,
                    in1=sub(st["bc"], 0, [[1, MACRO], [0, NHEADS]]),
                    op=ALU.mult)
                ge.tensor_tensor(
                    out=f_t, in0=f_t,
                    in1=sub(d_s.ap(), 0, [[0, MACRO], [1, NHEADS]]),
                    op=ALU.add)
                st["f"] = f_t

            def emit_b_y(st):
                t1_t, f_t = st["t1"], st["f"]
                yts_t = ytsp.tile([128, MACRO * D_INNER], BF16)
                yu_ts = []
                for q in range(8):
                    yu_t = yup.tile([128, 512], BF16)
                    ve.tensor_tensor(
                        out=yu_t,
                        in0=t1_t[:, q * 512:(q + 1) * 512],
                        in1=sub(f_t, q * 64,
                                [[NHEADS, 8], [0, 8], [1, NHEADS]]),
                        op=ALU.mult)
                    yu_ts.append(yu_t)
                st["yts"], st["yu"], st["sq"] = yts_t, yu_ts, {}

            def emit_b_ytr(st, q):
                yh = (q % 2) * 512
                for p in range(4):
                    nc.tensor.transpose(
                        ytp[:, yh + p * 128:yh + (p + 1) * 128],
                        st["yu"][q][:, p * 128:(p + 1) * 128],
                        id_s.ap())

            def emit_b_sqcp(st, q):
                yh = (q % 2) * 512
                if True:
                    ve.tensor_copy(st["yts"][:, q * 512:(q + 1) * 512],
                                   ytp[:, yh:yh + 512])
                else:
                    nc.scalar.copy(st["yts"][:, q * 512:(q + 1) * 512],
                                   ytp[:, yh:yh + 512])
                sq_t = sqp.tile([128, 512], BF16)
                ve.tensor_tensor(out=sq_t,
                                 in0=st["yts"][:, q * 512:(q + 1) * 512],
                                 in1=st["yts"][:, q * 512:(q + 1) * 512],
                                 op=ALU.mult)
                st["sq"][q] = sq_t

            def emit_b_ssred(st, q):
                nc.tensor.matmul(ssb[0:16, 0:512],
                                 essq_s[:, q * 16:(q + 1) * 16],
                                 st["sq"].pop(q),
                                 start=(q == 0), stop=(q == 7))

            def emit_b_tail(st):
                ssb_t = ssbp.tile([16, 512], F32)
                ve.tensor_copy(ssb_t, ssb[0:16, 0:512])
                for c in range(4):
                    nc.tensor.transpose(
                        ssb[:, 16 * c:16 * (c + 1)],
                        ssb_t[0:16, 128 * c:128 * (c + 1)],
                        idf_s.ap())
                # r column u = 16*c + 8*par + q  <->  block 8q + 2c + par
                r_t = rp.tile([128, MACRO], F32)
                nc.scalar.activation(r_t, ssb[:, 0:64], AF.Ln,
                                     scale=1.0 / 64, bias=eps_s.ap())
                nc.scalar.activation(r_t, r_t, AF.Exp, scale=-0.5)
                on_t = onp.tile([128, MACRO * D_MODEL], F32)
                st["r"], st["on"] = r_t, on_t
                del st["yu"]

            # ---- 5-stage macro pipeline, slot-interleaved emission ----
            g0s = []
            _g = 0
            for G in GROUPS:
                g0s.append(_g)
                _g += G
            for it in range(NM + 4):
                stA = states.get(it)
                if it < NM:
                    mb0 = it * MACRO
                    xt_t = xtp.tile([K1, MACRO * 128], BF16)
                    nc.sync.dma_start(xt_t,
                                      xt_d[:, mb0 * 128:
                                           (mb0 + MACRO) * 128])
                    t1_t = t1p.tile([128, MACRO * D_INNER], BF16)
                    bc_t = bcsp.tile([128, MACRO], F32)
                    stA = states[it] = {"mb0": mb0, "xt": xt_t,
                                        "t1": t1_t, "bc": bc_t}
                stB = states.get(it - 1)
                stC = states.get(it - 2)
                stE = states.get(it - 3)
                stN = states.get(it - 4)
                if stN is not None:
                    emit_c_norm(stN)
                    del states[it - 4]
                if stE is not None:
                    emit_c_exp(stE)
                if stB is not None:
                    emit_b_y(stB)
                for i in range(10):
                    if stB is not None:
                        if i < 8:
                            emit_b_ytr(stB, i)
                        if 1 <= i <= 8:
                            emit_b_sqcp(stB, i - 1)
                        if 2 <= i <= 9:
                            emit_b_ssred(stB, i - 2)
                    if stC is not None and i < 8:
                        emit_mm2_group(stC, i, part="mm")
                    if stC is not None and 1 <= i <= 8:
                        emit_mm2_group(stC, i - 1, part="on")
                    if stA is not None:
                        emit_a_mms(stA, i, g0s[i], GROUPS[i])
                        emit_a_group(stA, i, g0s[i], GROUPS[i])
                if stB is not None:
                    emit_b_tail(stB)
                if stA is not None:
                    emit_a_tail(stA)
    return nc


def split_overloaded_waits(nc, cap=1):
    n_fixed = 0
    for f in nc.m.functions:
        for bb in f.blocks:
            insts = bb.instructions
            i = 0
            while i < len(insts):
                ins = insts[i]
                si = ins.sync_info
                if si is not None and si.on_wait and len(si.on_wait) > cap:
                    waits = list(si.on_wait)
                    extra, keep = waits[:-cap], waits[-cap:]
                    pos = i
                    for j in range(0, len(extra), cap):
                        chunk = extra[j:j + cap]
                        nop = mybir.InstNoOp(
                            name=nc.get_next_instruction_name(), ins=[],
                            outs=[])
                        nop.engine = ins.engine
                        nop.sync_info = mybir.SyncInfo(on_wait=chunk,
                                                       on_update=[])
                        nc.register_instruction(nop)
                        insts.insert(pos, nop)
                        pos += 1
                        i += 1
                    si.on_wait = keep
                    ins.sync_info = si
                    n_fixed += 1
                i += 1
    return n_fixed


def run(x, f_out_w, f_out_b, in_proj_w, conv_w, conv_b, dt_bias, A_log,
        D_skip, norm_w, out_proj_w, num_cores=8, trace=False, sim_safe=False):
    from concourse.bass_utils import run_bass_kernel_spmd
    N = x.shape[0]
    assert N % (num_cores * MACRO * 128) == 0
    npc = N // num_cores
    NB = npc // 128
    w1t, woutt2, dsk = fold_weights(f_out_w, f_out_b, in_proj_w, conv_w,
                                    conv_b, dt_bias, A_log, D_skip, norm_w,
                                    out_proj_w)
    xt = prep_xt(x)
    essq, ident = make_consts()
    identf = np.eye(16, dtype=np.float32)
    nc = build_kernel(npc, num_cores=num_cores, sim_safe=sim_safe)
    split_overloaded_waits(nc)
    in_maps = []
    for c in range(num_cores):
        in_maps.append({
            "xt": np.ascontiguousarray(xt[:, c * npc:(c + 1) * npc]),
            "w1t": w1t, "woutt": woutt2, "dskip": dsk,
            "essq": essq, "ident": ident, "identf": identf,
            "eps": np.full((128, 1), NORM_EPS, np.float32),
        })
    res = run_bass_kernel_spmd(nc, in_maps, list(range(num_cores)),
                               trace=trace)
    outs = []
    for c in range(num_cores):
        o = np.asarray(res.results[c]["out"]).reshape(128, NB, D_MODEL)
        outs.append(np.ascontiguousarray(o.transpose(1, 0, 2))
                    .reshape(npc, D_MODEL))
    return np.concatenate(outs, axis=0).astype(np.float32), res


def kernel(x, f_out_w, f_out_b, in_proj_w, conv_w, conv_b, dt_bias, A_log,
           D_skip, norm_w, out_proj_w):
    # accept jax or numpy inputs; host-side folding needs mutable numpy
    args = [np.asarray(a) for a in
            (x, f_out_w, f_out_b, in_proj_w, conv_w, conv_b, dt_bias,
             A_log, D_skip, norm_w, out_proj_w)]
    out, _ = run(*args, num_cores=8)
    return out

